# revision 1
# baseline (speedup 1.0000x reference)
"""ColorHistogramLoss (soft histogram EMD) on 8 Trainium2 NeuronCores.

Strategy: pure data parallel over batch (B=8 -> one batch element per core).
Each core computes, for its 3 channels x {pred, target}, the 64-bin soft
(Gaussian-weighted) histogram of its 384x384 image:

    hist[j] = sum_px exp(-(x_px - c_j)^2 / denom)

The Gaussian is evaluated on the Scalar (ACT) engine via
Derivative_Erf(scale*x + bias) = 2/sqrt(pi)*exp(-u^2) with the fused
accum_out free-dim reduction.  ACT instruction overhead is large
(~0.3-0.6us), so instead of one instruction per (bin, image-pair) at
FD=2304 (192 instructions), each channel image is laid out [16, 9216] and
replicated 8x across partition blocks (host-side tile), and a
PER-PARTITION bias AP makes each 16-row block evaluate a different bin:
one instruction covers 8 bins x one image at FD=9216.  48 instructions
total (6 images x 8 bin-octets), ~10% faster end-to-end than the
192-instruction layout (measured 366-369us vs 402-405us per iteration,
same process).  An on-device step-0 replicating DMA was tried and is
faster to feed, but intermittently corrupts results and can wedge the
device (NRT_EXEC_UNIT_UNRECOVERABLE) — do not reintroduce it.

Per-partition partial sums land in Hacc[128, 48]; one PE matmul against an
8-column block selector reduces over the 16 rows of each block, giving all
384 histogram values as [48, 8] in PSUM.  The tiny tail (normalize,
cumsum, |diff|, mean over 8*3*64) runs on host in float64.
"""

import functools
import math

import numpy as np

N_CORES = 8
NUM_BINS = 64
B, C, H, W = 8, 3, 384, 384
HW = H * W
N_UNITS = 2 * C                 # (channel, pred/target) images per core
N_OCT = NUM_BINS // 8           # 8 bin-octets; one ACT instruction each
FREE = HW // 16                 # channel image as [16, 9216]
DENOM = 2.0 * (1.0 / 64.0) ** 2 + 1e-7
SCALE = 1.0 / math.sqrt(DENOM)
DERF_SCALE = math.sqrt(math.pi) / 2.0  # Derivative_Erf = 2/sqrt(pi) * exp(-u^2)


def _build_program():
    import concourse.bass as bass
    import concourse.mybir as mybir

    nc = bass.Bass()
    xs = [
        nc.dram_tensor(f"x{u}", [128, FREE], mybir.dt.float32, kind="ExternalInput")
        for u in range(N_UNITS)
    ]
    cst = nc.dram_tensor("consts", [128, 16], mybir.dt.float32, kind="ExternalInput")
    hist_out = nc.dram_tensor(
        "hist", [N_UNITS * N_OCT, 8], mybir.dt.float32, kind="ExternalOutput"
    )

    with (
        nc.sbuf_tensor("xt0", [128, FREE], mybir.dt.float32) as xt0,
        nc.sbuf_tensor("xt1", [128, FREE], mybir.dt.float32) as xt1,
        nc.sbuf_tensor("xt2", [128, FREE], mybir.dt.float32) as xt2,
        nc.sbuf_tensor("cstt", [128, 16], mybir.dt.float32) as cstt,
        nc.sbuf_tensor("wscr", [128, FREE], mybir.dt.float32) as wscr,
        nc.sbuf_tensor("hacc", [128, N_UNITS * N_OCT], mybir.dt.float32) as hacc,
        nc.sbuf_tensor("ho", [N_UNITS * N_OCT, 8], mybir.dt.float32) as ho,
        nc.psum_tensor("ph", [N_UNITS * N_OCT, 8], mybir.dt.float32) as ph,
        nc.semaphore("sem_c") as sem_c,
        nc.semaphore("sem_x0") as sem_x0,
        nc.semaphore("sem_x1") as sem_x1,
        nc.semaphore("sem_x2") as sem_x2,
        nc.semaphore("act_sem") as act_sem,
        nc.semaphore("pe_sem") as pe_sem,
        nc.semaphore("cp_sem") as cp_sem,
        nc.Block() as block,
    ):
        slots = [xt0, xt1, xt2]
        xsems = [sem_x0, sem_x1, sem_x2]

        @block.sync
        def _(sync):
            sync.dma_start(out=cstt[:], in_=cst[:]).then_inc(sem_c, 16)
            for u in range(N_UNITS):
                slot = u % 3
                if u >= 3:
                    # slot is free once unit u-3's 8 ACT instructions are done
                    sync.wait_ge(act_sem, N_OCT * (u - 2))
                sync.dma_start(out=slots[slot][:], in_=xs[u][:]).then_inc(
                    xsems[slot], 16
                )
            sync.wait_ge(cp_sem, 1)
            sync.dma_start(out=hist_out[:], in_=ho[:]).then_inc(sem_c, 16)

        @block.scalar
        def _(scalar):
            # dummy activation on scratch: pulls the ACT table load (~2.7us)
            # forward so it overlaps with the input DMAs
            scalar.activation(
                wscr[0:128, 0:1], wscr[0:128, 1:2],
                mybir.ActivationFunctionType.Derivative_Erf,
                bias=wscr[:, 2:3], scale=1.0,
            )
            scalar.wait_ge(sem_c, 16)
            for u in range(N_UNITS):
                slot = u % 3
                scalar.wait_ge(xsems[slot], 16 * (u // 3 + 1))
                for o in range(N_OCT):
                    # partition block k (rows 16k..16k+15) evaluates bin 8o+k
                    scalar.activation(
                        wscr[:],
                        slots[slot][:],
                        mybir.ActivationFunctionType.Derivative_Erf,
                        bias=cstt[:, o : o + 1],
                        scale=float(SCALE),
                        accum_out=hacc[:, N_OCT * u + o : N_OCT * u + o + 1],
                    ).then_inc(act_sem, 1)

        @block.tensor
        def _(tensor):
            tensor.wait_ge(act_sem, N_UNITS * N_OCT)
            # ph[col, k] = sum_p hacc[p, col] * sel[p, k]  (sel: p//16 == k)
            tensor.matmul(
                ph[0 : N_UNITS * N_OCT, 0:8],
                hacc[:, :],
                cstt[:, 8:16],
                start=True,
                stop=True,
            ).then_inc(pe_sem, 1)

        @block.vector
        def _(vector):
            vector.wait_ge(pe_sem, 1)
            vector.tensor_copy(ho[:, :], ph[:, :]).then_inc(cp_sem, 1)

    return nc


def _make_consts():
    centers = np.linspace(0.0, 1.0, NUM_BINS, dtype=np.float32)
    bias = (-centers.astype(np.float64) * SCALE).astype(np.float32)
    cst = np.zeros((128, 16), dtype=np.float32)
    p = np.arange(128)
    for o in range(N_OCT):
        cst[:, o] = bias[8 * o + p // 16]      # per-partition bias: block k -> bin 8o+k
    for k in range(8):
        cst[p // 16 == k, 8 + k] = 1.0         # block selector for the PE reduce
    return cst


@functools.lru_cache(maxsize=1)
def _get_runner():
    """Compile the SPMD program once; return a callable list[in_map] -> list[out_map]."""
    import jax
    from jax.experimental.shard_map import shard_map
    from jax.sharding import Mesh, PartitionSpec

    from concourse import mybir
    from concourse.bass2jax import (
        _bass_exec_p,
        install_neuronx_cc_hook,
        partition_id_tensor,
    )

    nc = _build_program()
    install_neuronx_cc_hook()

    partition_name = (
        nc.partition_id_tensor.name if nc.partition_id_tensor else None
    )
    in_names, out_names, out_avals, zero_outs = [], [], [], []
    for alloc in nc.m.functions[0].allocations:
        if not isinstance(alloc, mybir.MemoryLocationSet):
            continue
        name = alloc.memorylocations[0].name
        if alloc.kind == "ExternalInput":
            if name != partition_name:
                in_names.append(name)
        elif alloc.kind == "ExternalOutput":
            out_names.append(name)
            shape = tuple(alloc.tensor_shape)
            dtype = mybir.dt.np(alloc.dtype)
            out_avals.append(jax.core.ShapedArray(shape, dtype))
            zero_outs.append(np.zeros(shape, dtype))
    n_params = len(in_names)
    n_outs = len(out_avals)
    all_in_names = list(in_names) + list(out_names)
    if partition_name is not None:
        all_in_names.append(partition_name)
    donate = tuple(range(n_params, n_params + n_outs))

    def _body(*args):
        operands = list(args)
        if partition_name is not None:
            operands.append(partition_id_tensor())
        outs = _bass_exec_p.bind(
            *operands,
            out_avals=tuple(out_avals),
            in_names=tuple(all_in_names),
            out_names=tuple(out_names),
            lowering_input_output_aliases=(),
            sim_require_finite=True,
            sim_require_nnan=True,
            nc=nc,
        )
        return tuple(outs)

    devices = jax.devices()[:N_CORES]
    mesh = Mesh(np.asarray(devices), ("core",))
    sharded = jax.jit(
        shard_map(
            _body,
            mesh=mesh,
            in_specs=(PartitionSpec("core"),) * (n_params + n_outs),
            out_specs=(PartitionSpec("core"),) * n_outs,
            check_rep=False,
        ),
        donate_argnums=donate,
        keep_unused=True,
    )

    class Runner:
        def __init__(self):
            self.sharded = sharded
            self.in_names = in_names
            self.out_names = out_names
            self.out_avals = out_avals
            self.zero_outs = zero_outs

        def concat_inputs(self, in_maps):
            return [
                np.concatenate([np.asarray(m[name]) for m in in_maps], axis=0)
                for name in in_names
            ]

        def fresh_zeros(self):
            return [
                np.zeros((N_CORES * z.shape[0], *z.shape[1:]), z.dtype)
                for z in zero_outs
            ]

        def split_outputs(self, out_arrs):
            return [
                {
                    name: np.asarray(out_arrs[i]).reshape(
                        N_CORES, *out_avals[i].shape
                    )[c]
                    for i, name in enumerate(out_names)
                }
                for c in range(N_CORES)
            ]

        def __call__(self, in_maps):
            out_arrs = self.sharded(*self.concat_inputs(in_maps), *self.fresh_zeros())
            return self.split_outputs(out_arrs)

    return Runner()


def _shard_inputs(pred, target):
    cst = _make_consts()
    maps = []
    for b in range(B):
        m = {"consts": cst}
        for c in range(C):
            for t, src in enumerate((pred, target)):
                u = 2 * c + t
                img = np.ascontiguousarray(src[b, c], dtype=np.float32).reshape(
                    16, FREE
                )
                m[f"x{u}"] = np.tile(img, (8, 1))
        maps.append(m)
    return maps


def _finish_on_host(results):
    total = 0.0
    for b in range(B):
        hist = results[b]["hist"].astype(np.float64) * DERF_SCALE
        for c in range(C):
            p = hist[N_OCT * (2 * c) : N_OCT * (2 * c) + N_OCT, :].reshape(NUM_BINS)
            t = hist[N_OCT * (2 * c + 1) : N_OCT * (2 * c + 1) + N_OCT, :].reshape(
                NUM_BINS
            )
            pn = p / (p.sum() + 1e-7)
            tn = t / (t.sum() + 1e-7)
            total += np.abs(np.cumsum(pn) - np.cumsum(tn)).sum()
    return np.float32(total / (B * C * NUM_BINS))


def kernel(pred, target):
    pred = np.asarray(pred, dtype=np.float32)
    target = np.asarray(target, dtype=np.float32)
    assert pred.shape == (B, C, H, W) and target.shape == (B, C, H, W)
    run = _get_runner()
    results = run(_shard_inputs(pred, target))
    return np.asarray(_finish_on_host(results), dtype=np.float32)

